# revision 2
# baseline (speedup 1.0000x reference)
"""GaussianPolicy (LIF spiking encoder + twin MLP heads) on 8 TRN2 cores.

Data-parallel: batch 4096 -> 512 per core. Per-core layout keeps the
hidden dim on SBUF partitions and batch on the free dim, so every GEMM is
out[h,b] = W^T-tile.T @ rhs[k,b] with weights stationary.  Biases are
folded in as an extra K=1 matmul row against a ones vector.  The LIF scan
runs on DVE with fused scalar_tensor_tensor ops (4 ops/step).
"""

import numpy as np
from contextlib import ExitStack

import concourse.bass as bass
import concourse.tile as tile
from concourse import bacc, mybir
from concourse.bass_utils import run_bass_kernel_spmd

try:
    import ml_dtypes

    BF16_NP = ml_dtypes.bfloat16
except Exception:  # pragma: no cover
    BF16_NP = None

P = 128
B, IN, H, A = 4096, 512, 2048, 32
NCORES = 8
BC = B // NCORES          # 512 batch rows per core
TU, REP = 5, 3            # 5 unique timesteps replicated 3x -> 15
T = TU * REP
NH = H // P               # 16 hidden tiles
NI = IN // P              # 4 input k-tiles
DECAY, THRESH = 0.2, 0.2
LOG_SIG_MIN, LOG_SIG_MAX = -20.0, 2.0

F32 = mybir.dt.float32
BF16 = mybir.dt.bfloat16
FC_DT = F32     # fc GEMM precision (protects the spike threshold)
MLP_DT = BF16   # hidden/head GEMM precision

OP = mybir.AluOpType
AF = mybir.ActivationFunctionType


def _build_nc():
    nc = bacc.Bacc(None, target_bir_lowering=False, debug=False)

    stateT = nc.dram_tensor("stateT", [TU, IN, BC], FC_DT, kind="ExternalInput")
    wlifT = nc.dram_tensor("wlifT", [IN + 1, H], FC_DT, kind="ExternalInput")
    w11T = nc.dram_tensor("w11T", [H + 1, H], MLP_DT, kind="ExternalInput")
    w12T = nc.dram_tensor("w12T", [H + 1, H], MLP_DT, kind="ExternalInput")
    w21T = nc.dram_tensor("w21T", [H + 1, H], MLP_DT, kind="ExternalInput")
    w22T = nc.dram_tensor("w22T", [H + 1, H], MLP_DT, kind="ExternalInput")
    wmT = nc.dram_tensor("wmT", [H + 1, A], MLP_DT, kind="ExternalInput")
    wlsT = nc.dram_tensor("wlsT", [H + 1, A], MLP_DT, kind="ExternalInput")
    mean_o = nc.dram_tensor("mean_o", [A, BC], F32, kind="ExternalOutput")
    ls_o = nc.dram_tensor("ls_o", [A, BC], F32, kind="ExternalOutput")

    with tile.TileContext(nc) as tc, ExitStack() as ctx:
        cpool = ctx.enter_context(tc.tile_pool(name="consts", bufs=1))
        spool = ctx.enter_context(tc.tile_pool(name="state", bufs=TU * NI))
        wfpool = ctx.enter_context(tc.tile_pool(name="wf", bufs=8))
        bfpool = ctx.enter_context(tc.tile_pool(name="bf", bufs=4))
        fcpool = ctx.enter_context(tc.tile_pool(name="fc", bufs=2))
        scpool = ctx.enter_context(tc.tile_pool(name="scan", bufs=2))
        xpool = ctx.enter_context(tc.tile_pool(name="x", bufs=1))
        apool = ctx.enter_context(tc.tile_pool(name="acts", bufs=2))
        wbpool = ctx.enter_context(tc.tile_pool(name="wb", bufs=16))
        bbpool = ctx.enter_context(tc.tile_pool(name="bb", bufs=4))
        hpool = ctx.enter_context(tc.tile_pool(name="hw", bufs=4))
        opool = ctx.enter_context(tc.tile_pool(name="outs", bufs=2))
        pspool = ctx.enter_context(
            tc.tile_pool(name="ps", bufs=4, space=bass.MemorySpace.PSUM)
        )
        pshead = ctx.enter_context(
            tc.tile_pool(name="psh", bufs=2, space=bass.MemorySpace.PSUM)
        )

        ones_f = cpool.tile([1, BC], FC_DT, tag="ones_f")
        nc.vector.memset(ones_f[:], 1.0)
        ones_b = cpool.tile([1, BC], MLP_DT, tag="ones_b")
        nc.vector.memset(ones_b[:], 1.0)

        # resident state tiles [i=128, b=512] per (t, k)
        st = {}
        for t in range(TU):
            for k in range(NI):
                s = spool.tile([P, BC], FC_DT, tag="st")
                nc.sync.dma_start(out=s[:], in_=stateT[t, k * P:(k + 1) * P, :])
                st[(t, k)] = s

        # x_all holds the per-batch spike counts (0..15) in f32, xb in MLP_DT
        x_all = xpool.tile([P, NH, BC], F32, tag="x_all")
        xb_all = xpool.tile([P, NH, BC], MLP_DT, tag="xb_all")

        # ---- Phase 1: fc GEMM + LIF scan, one hidden tile at a time ----
        for j in range(NH):
            wk = []
            for k in range(NI):
                w = wfpool.tile([P, P], FC_DT, tag="wf")
                nc.sync.dma_start(
                    out=w[:], in_=wlifT[k * P:(k + 1) * P, j * P:(j + 1) * P]
                )
                wk.append(w)
            brow = bfpool.tile([1, P], FC_DT, tag="bf")
            nc.sync.dma_start(out=brow[:], in_=wlifT[IN:IN + 1, j * P:(j + 1) * P])

            fc = fcpool.tile([P, TU, BC], F32, tag="fc")
            for t in range(TU):
                ps = pspool.tile([P, BC], F32, tag="ps")
                for k in range(NI):
                    nc.tensor.matmul(
                        ps[:], wk[k][:], st[(t, k)][:], start=(k == 0), stop=False
                    )
                nc.tensor.matmul(ps[:], brow[:], ones_f[:], start=False, stop=True)
                nc.scalar.activation(fc[:, t, :], ps[:], AF.Copy)

            # LIF scan: mem' = DECAY*mem*(mem<=TH) + fc_t ; count spikes
            x_sl = x_all[:, j, :]
            mem = scpool.tile([P, BC], F32, tag="mem")
            tmp = scpool.tile([P, BC], F32, tag="tmp")
            nc.vector.tensor_scalar(x_sl, fc[:, 0, :], THRESH, None, op0=OP.is_gt)
            mem_src = fc[:, 0, :]
            for t in range(1, T):
                fct = fc[:, t // REP, :]
                nc.vector.tensor_scalar(tmp[:], mem_src, THRESH, None, op0=OP.is_le)
                nc.vector.tensor_tensor(tmp[:], mem_src, tmp[:], op=OP.mult)
                nc.vector.scalar_tensor_tensor(
                    mem[:], tmp[:], DECAY, fct, op0=OP.mult, op1=OP.add
                )
                nc.vector.scalar_tensor_tensor(
                    x_sl, mem[:], THRESH, x_sl, op0=OP.is_gt, op1=OP.add
                )
                mem_src = mem[:]
            # bf16 copy for the MLP GEMMs (counts <= 15 are exact in bf16)
            nc.scalar.activation(xb_all[:, j, :], x_sl, AF.Copy)

        # ---- Phase 2: hidden layers (streamed weights, bias via ones row) ----
        def dense(w_dram, src, relu, out_dt):
            dst = apool.tile([P, NH, BC], out_dt, tag="act")
            for jo in range(NH):
                ps = pspool.tile([P, BC], F32, tag="ps")
                for k in range(NH):
                    w = wbpool.tile([P, P], MLP_DT, tag="wb")
                    nc.sync.dma_start(
                        out=w[:], in_=w_dram[k * P:(k + 1) * P, jo * P:(jo + 1) * P]
                    )
                    nc.tensor.matmul(
                        ps[:], w[:], src[:, k, :], start=(k == 0), stop=False
                    )
                brow = bbpool.tile([1, P], MLP_DT, tag="bb")
                nc.sync.dma_start(out=brow[:], in_=w_dram[H:H + 1, jo * P:(jo + 1) * P])
                nc.tensor.matmul(ps[:], brow[:], ones_b[:], start=False, stop=True)
                nc.scalar.activation(
                    dst[:, jo, :], ps[:], AF.Relu if relu else AF.Copy
                )
            return dst

        def head(w_dram, src):
            ps = pshead.tile([A, BC], F32, tag="psh")
            for k in range(NH):
                w = hpool.tile([P, A], MLP_DT, tag="hw")
                nc.sync.dma_start(out=w[:], in_=w_dram[k * P:(k + 1) * P, :])
                nc.tensor.matmul(ps[:], w[:], src[:, k, :], start=(k == 0), stop=False)
            brow = hpool.tile([1, A], MLP_DT, tag="hb")
            nc.sync.dma_start(out=brow[:], in_=w_dram[H:H + 1, :])
            nc.tensor.matmul(ps[:], brow[:], ones_b[:], start=False, stop=True)
            return ps

        x1 = dense(w11T, xb_all, True, MLP_DT)
        x1b = dense(w12T, x1, True, MLP_DT)
        ps_m = head(wmT, x1b)
        m_s = opool.tile([A, BC], F32, tag="mo")
        nc.scalar.activation(m_s[:], ps_m[:], AF.Copy)
        nc.sync.dma_start(out=mean_o[:], in_=m_s[:])

        x2 = dense(w21T, xb_all, True, MLP_DT)
        x2b = dense(w22T, x2, True, MLP_DT)
        ps_l = head(wlsT, x2b)
        l_s = opool.tile([A, BC], F32, tag="lo")
        nc.vector.tensor_scalar(
            l_s[:], ps_l[:], LOG_SIG_MIN, LOG_SIG_MAX, op0=OP.max, op1=OP.min
        )
        nc.sync.dma_start(out=ls_o[:], in_=l_s[:])

    nc.compile()
    return nc


_NC_CACHE = None


def kernel(state, W_lif, b_lif, W11, b11, W12, b12, W21, b21, W22, b22,
           Wm, bm, Wls, bls):
    global _NC_CACHE
    if _NC_CACHE is None:
        _NC_CACHE = _build_nc()
    nc = _NC_CACHE

    f32 = np.float32
    state = np.asarray(state, f32)

    def ext_f(wT, b):  # [K+1, M] f32
        return np.ascontiguousarray(
            np.vstack([np.asarray(wT, f32), np.asarray(b, f32)[None, :]])
        )

    def ext_b(wT, b, scale=1.0):  # [K+1, M] bf16, optional src scaling
        m = np.vstack(
            [np.asarray(wT, f32) * scale, np.asarray(b, f32)[None, :]]
        )
        return np.ascontiguousarray(m.astype(BF16_NP))

    wlif_e = ext_f(np.asarray(W_lif, f32).T, b_lif)
    # mean over 15 steps folded into the first-layer weights
    w11_e = ext_b(np.asarray(W11, f32).T, b11, 1.0 / T)
    w12_e = ext_b(np.asarray(W12, f32).T, b12)
    w21_e = ext_b(np.asarray(W21, f32).T, b21, 1.0 / T)
    w22_e = ext_b(np.asarray(W22, f32).T, b22)
    wm_e = ext_b(np.asarray(Wm, f32).T, bm)
    wls_e = ext_b(np.asarray(Wls, f32).T, bls)

    in_maps = []
    for c in range(NCORES):
        sh = state[c * BC:(c + 1) * BC]            # [BC, 5, IN]
        stateT = np.ascontiguousarray(sh.transpose(1, 2, 0))  # [5, IN, BC]
        in_maps.append({
            "stateT": stateT,
            "wlifT": wlif_e,
            "w11T": w11_e, "w12T": w12_e,
            "w21T": w21_e, "w22T": w22_e,
            "wmT": wm_e, "wlsT": wls_e,
        })

    res = run_bass_kernel_spmd(nc, in_maps, core_ids=list(range(NCORES))).results
    mean = np.concatenate(
        [np.asarray(res[c]["mean_o"], f32).T for c in range(NCORES)], axis=0
    )
    log_std = np.concatenate(
        [np.asarray(res[c]["ls_o"], f32).T for c in range(NCORES)], axis=0
    )
    return mean, log_std
